# revision 1
# baseline (speedup 1.0000x reference)
"""Trainium2 Bass kernel v2 for the ExplicitV2C GNN layer (GATv2).

Strategy (8-core SPMD):
  * Host: assign nodes to 512 degree-balanced bins of 128; per core, order its
    64 bins by subtile count (desc) so all cores share one per-window subtile
    profile S_prof (SPMD-identical program, core-specific data).
  * Device per core (all matmuls bf16, PSUM f32):
      Phase 1 (nodes, 4 tiles/iter): LLR fusion. LN stats via bn_stats;
        normalization via per-tile ACT scale/bias; gamma/beta+relu folded into
        the feature-major ACT copy after transpose; mask select via
        copy_predicated against a PE-broadcast mask.
      AllGather x_w table in bf16.
      Phase 2 (edges): per-subtile indirect gathers of x_w[src] rows; one-hot
        dst matrices via is_equal (edge-major) and PE ones-broadcast of the
        dst row (node-major); GATv2 scores with att.z folded into small
        matmuls + att.relu(-z) on ACT; segment softmax + aggregation via
        one-hot matmuls accumulated in PSUM.
      Phase 3 (nodes, 4 tiles/iter): degree gate + final LN with batched
        sqrt passes (2 activation-table loads total).
  * Host: concatenate shards, undo the node permutation.
"""

import os
import sys

sys.path.insert(0, "/opt/trn_rl_repo")

import numpy as np
import ml_dtypes

import concourse.bass as bass
import concourse.bacc as bacc
import concourse.mybir as mybir
import concourse.tile as tile
from concourse.bass import IndirectOffsetOnAxis
from concourse.bass_utils import run_bass_kernel_spmd

F32 = mybir.dt.float32
BF16 = mybir.dt.bfloat16
I32 = mybir.dt.int32
AX = mybir.AxisListType
OP = mybir.AluOpType
AF = mybir.ActivationFunctionType

P = 128
NCORES = 8
LN_EPS = 1e-5
SM_EPS = 1e-16
BF = ml_dtypes.bfloat16


class Cfg2:
    def __init__(self, N, E, S_prof, chunks):
        self.N, self.E = N, E
        self.BINS = N // P
        self.BPC = self.BINS // NCORES      # windows per core
        self.NSHARD = N // NCORES
        self.S_prof = tuple(S_prof)         # per-window subtile counts
        self.offs = tuple(np.concatenate([[0], np.cumsum(S_prof)]).tolist())
        self.TOTSUB = int(self.offs[-1])
        self.chunks = tuple(chunks)         # list of (w0, w1) window ranges
        self.B1 = 4                         # node tiles per phase1/3 group
        assert self.BPC % self.B1 == 0

    def key(self):
        return (self.N, self.E, self.S_prof, self.chunks)


# ----------------------------------------------------------------------------
# Host-side preprocessing
# ----------------------------------------------------------------------------

def _balance_bins(deg_in, N, BINS, target):
    order = np.argsort(-deg_in, kind="stable")
    bin_of = np.empty(N, np.int64)
    slot_of = np.empty(N, np.int64)
    bin_of[order] = np.arange(N) % BINS
    slot_of[order] = np.arange(N) // BINS
    loads = np.bincount(bin_of, weights=deg_in, minlength=BINS).astype(np.int64)

    for _ in range(4000):
        a = int(np.argmax(loads))
        if loads[a] <= target:
            break
        b = int(np.argmin(loads))
        nodes_a = np.where(bin_of == a)[0]
        nodes_b = np.where(bin_of == b)[0]
        da = deg_in[nodes_a]
        db = deg_in[nodes_b]
        need = loads[a] - target
        best = None
        du_max = da.max()
        for u_idx in np.argsort(-da)[:8]:
            u = nodes_a[u_idx]
            du = deg_in[u]
            want = du - need
            v_idx = int(np.argmin(np.abs(db - max(want, 0))))
            v = nodes_b[v_idx]
            dv = deg_in[v]
            if du <= dv:
                continue
            gain = du - dv
            if loads[b] + gain > target + du_max:
                continue
            best = (u, v, gain)
            break
        if best is None:
            break
        u, v, gain = best
        bu, su = bin_of[u], slot_of[u]
        bin_of[u], slot_of[u] = bin_of[v], slot_of[v]
        bin_of[v], slot_of[v] = bu, su
        loads[a] -= gain
        loads[b] += gain
    return bin_of, slot_of, loads


def host_prep(inputs, N=65536, E=262144):
    BINS = N // P
    BPC = BINS // NCORES
    NSHARD = N // NCORES

    x = np.asarray(inputs["x"], np.float32)
    ei = np.asarray(inputs["edge_index"])
    src_o = ei[0].astype(np.int64)
    dst_o = ei[1].astype(np.int64)
    ea = np.asarray(inputs["edge_attr"], np.float32)
    ndeg = np.asarray(inputs["node_degrees"]).astype(np.int64)
    llr = np.asarray(inputs["llr_features"], np.float32).reshape(N)
    vmask = np.asarray(inputs["var_node_mask"]).astype(np.float32).reshape(N)

    deg_in = np.bincount(dst_o, minlength=N).astype(np.int64)
    target = -(-E // BINS)
    bin_of, slot_of, loads = _balance_bins(deg_in, N, BINS, target)
    S_bin = np.maximum(1, -(-loads // P))          # subtiles needed per bin

    # per-core window ordering: sort own bins by S desc -> shared S profile
    win_of = np.empty((NCORES, BPC), np.int64)     # win_of[c, w] = bin id
    S_mat = np.empty((NCORES, BPC), np.int64)
    for c in range(NCORES):
        bins_c = np.arange(c * BPC, (c + 1) * BPC)
        order = bins_c[np.argsort(-S_bin[bins_c], kind="stable")]
        win_of[c] = order
        S_mat[c] = S_bin[order]
    S_prof = S_mat.max(axis=0)
    offs = np.concatenate([[0], np.cumsum(S_prof)])
    TOTSUB = int(offs[-1])

    # chunk windows into ~8 groups of equal subtile totals
    NCH = 8
    tgt = TOTSUB / NCH
    chunks = []
    w0 = 0
    acc = 0.0
    for w in range(BPC):
        acc += S_prof[w]
        if acc >= tgt * (len(chunks) + 1) - 1e-9 and w + 1 > w0:
            chunks.append((w0, w + 1))
            w0 = w + 1
            if len(chunks) == NCH - 1:
                break
    chunks.append((w0, BPC))
    cfg = Cfg2(N, E, S_prof.tolist(), chunks)

    # node -> (core, window) and global permuted id
    core_of_bin = np.empty(BINS, np.int64)
    win_of_bin = np.empty(BINS, np.int64)
    for c in range(NCORES):
        core_of_bin[win_of[c]] = c
        win_of_bin[win_of[c]] = np.arange(BPC)
    nc_core = core_of_bin[bin_of]
    nc_win = win_of_bin[bin_of]
    o2p = nc_core * NSHARD + nc_win * P + slot_of
    p2o = np.argsort(o2p)

    # --- edge arrays -----------------------------------------------------------
    src_p = o2p[src_o].astype(np.int32)
    e_core = nc_core[dst_o]
    e_win = nc_win[dst_o]
    e_slot = slot_of[dst_o]

    idx_h = np.zeros((NCORES, P, TOTSUB), np.int32)
    dst_h = np.full((NCORES, P, TOTSUB), float(P), BF)
    ea_h = np.zeros((NCORES, 8, TOTSUB * P), BF)

    key = e_core * BPC + e_win
    eorder = np.argsort(key, kind="stable")
    key_s = key[eorder]
    starts = np.zeros(NCORES * BPC + 1, np.int64)
    np.cumsum(np.bincount(key_s, minlength=NCORES * BPC), out=starts[1:])
    rank = np.arange(E) - starts[key_s]
    ec = e_core[eorder]
    ew = e_win[eorder]
    sub = offs[ew] + rank // P                     # global subtile index
    pp = rank % P
    idx_h[ec, pp, sub] = src_p[eorder]
    dst_h[ec, pp, sub] = e_slot[eorder].astype(np.float32)
    ea_h[ec, :, sub * P + pp] = ea[eorder].astype(BF)

    dstr_h = dst_h.transpose(0, 2, 1).reshape(NCORES, 1, TOTSUB * P)

    # --- node arrays -----------------------------------------------------------
    xp = x[p2o]
    x_t = xp.reshape(NCORES, NSHARD, P).transpose(0, 2, 1).astype(BF)
    lm = np.stack(
        [llr[p2o].reshape(NCORES, NSHARD), vmask[p2o].reshape(NCORES, NSHARD),
         np.ones((NCORES, NSHARD), np.float32)],
        axis=1).astype(BF)
    degc = np.clip(ndeg, 0, 99)[p2o].reshape(NCORES, BPC, P)
    deg_h = degc.transpose(0, 2, 1).astype(np.int32)  # [c, P, BPC]

    # --- weights ---------------------------------------------------------------
    w = {k: np.asarray(v, np.float32) for k, v in inputs.items()
         if k not in ("x", "edge_index", "edge_attr", "node_degrees",
                      "llr_features", "var_node_mask")}
    att = w["att"]
    H = att.shape[0]
    consts = {
        "c_Wfx": w["W_f"][:P].astype(BF),
        "c_wfl": w["W_f"][P:P + 1].astype(BF),
        "c_bfr": w["b_f"].reshape(1, P).astype(BF),
        "c_gfc": w["g_f"].reshape(P, 1).astype(np.float32),
        "c_befc": w["be_f"].reshape(P, 1).astype(np.float32),
        "c_Wl": w["W_l"].astype(BF),
        "c_Wr": w["W_r"].astype(BF),
        "c_We": w["W_e"].astype(BF),
        "c_attb": np.broadcast_to(att.reshape(1, H * P), (P, H * P)).astype(BF),
        "c_wla": (w["W_l"].reshape(P, H, P) * att[None]).sum(-1).astype(BF),
        "c_wra": (w["W_r"].reshape(P, H, P) * att[None]).sum(-1).astype(BF),
        "c_wea": (w["W_e"].reshape(8, H, P) * att[None]).sum(-1).astype(BF),
        "c_iota": np.broadcast_to(
            np.arange(P, dtype=np.float32)[None, :], (P, P)).astype(BF),
        "c_iotap": np.arange(P, dtype=np.float32).reshape(P, 1),
        "c_identb": np.eye(P, dtype=np.float32).astype(BF),
        "c_ones1": np.ones((1, P), BF),
        "c_Wg1x": w["W_g1"][:P].astype(BF),
        "c_Wg2": w["W_g2"].astype(BF),
        "c_bg2r": w["b_g2"].reshape(1, P).astype(BF),
        "c_gg": w["g_g"].reshape(P, 1).astype(np.float32),
        "c_beg": w["be_g"].reshape(P, 1).astype(np.float32),
        "c_gob": np.broadcast_to(w["g_o"].reshape(1, P), (P, P)).astype(np.float32),
        "c_bob": np.broadcast_to(w["b_o"].reshape(1, P), (P, P)).astype(np.float32),
        "c_Td": (w["deg_emb"] @ w["W_g1"][P:P + 16] + w["b_g1"][None, :]).astype(BF),
    }
    consts = {k: np.ascontiguousarray(v) for k, v in consts.items()}

    in_maps = []
    for c in range(NCORES):
        m = {
            "x_t": np.ascontiguousarray(x_t[c]),
            "lm": np.ascontiguousarray(lm[c]),
            "e_idx": np.ascontiguousarray(idx_h[c]),
            "e_dstc": np.ascontiguousarray(dst_h[c]),
            "e_dstr": np.ascontiguousarray(dstr_h[c]),
            "e_att": np.ascontiguousarray(ea_h[c]),
            "deg_i": np.ascontiguousarray(deg_h[c]),
        }
        m.update(consts)
        in_maps.append(m)
    return cfg, in_maps, p2o


# ----------------------------------------------------------------------------
# Device kernel
# ----------------------------------------------------------------------------

CSHAPE = {
    "c_Wfx": ([P, P], BF16), "c_wfl": ([1, P], BF16), "c_bfr": ([1, P], BF16),
    "c_gfc": ([P, 1], F32), "c_befc": ([P, 1], F32),
    "c_Wl": ([P, 512], BF16), "c_Wr": ([P, 512], BF16), "c_We": ([8, 512], BF16),
    "c_attb": ([P, 512], BF16),
    "c_wla": ([P, 4], BF16), "c_wra": ([P, 4], BF16), "c_wea": ([8, 4], BF16),
    "c_iota": ([P, P], BF16), "c_iotap": ([P, 1], F32),
    "c_identb": ([P, P], BF16), "c_ones1": ([1, P], BF16),
    "c_Wg1x": ([P, P], BF16), "c_Wg2": ([P, P], BF16), "c_bg2r": ([1, P], BF16),
    "c_gg": ([P, 1], F32), "c_beg": ([P, 1], F32),
    "c_gob": ([P, P], F32), "c_bob": ([P, P], F32),
    "c_Td": ([100, P], BF16),
}


def _bc(ap, shape):
    return ap.to_broadcast(shape)


def build_kernel(cfg):
    N, BPC, NSHARD = cfg.N, cfg.BPC, cfg.NSHARD
    S_prof, offs, TOTSUB = cfg.S_prof, cfg.offs, cfg.TOTSUB
    B1, NG1 = cfg.B1, cfg.BPC // cfg.B1
    W1 = B1 * P
    PHASES = int(os.environ.get("GNN_PHASES", "3"))
    SKIP_AG = bool(int(os.environ.get("GNN_SKIP_AG", "0")))
    REPS = int(os.environ.get("GNN_BODY_REPS", "1"))

    nc = bacc.Bacc("TRN2", target_bir_lowering=False, debug=False,
                   num_devices=NCORES)

    d_xt = nc.dram_tensor("x_t", [P, NSHARD], BF16, kind="ExternalInput")
    d_lm = nc.dram_tensor("lm", [3, NSHARD], BF16, kind="ExternalInput")
    d_idx = nc.dram_tensor("e_idx", [P, TOTSUB], I32, kind="ExternalInput")
    d_dstc = nc.dram_tensor("e_dstc", [P, TOTSUB], BF16, kind="ExternalInput")
    d_dstr = nc.dram_tensor("e_dstr", [1, TOTSUB * P], BF16, kind="ExternalInput")
    d_eat = nc.dram_tensor("e_att", [8, TOTSUB * P], BF16, kind="ExternalInput")
    d_deg = nc.dram_tensor("deg_i", [P, BPC], I32, kind="ExternalInput")
    d_out = nc.dram_tensor("y", [NSHARD, P], F32, kind="ExternalOutput")
    d_c = {k: nc.dram_tensor(k, sh, dt, kind="ExternalInput")
           for k, (sh, dt) in CSHAPE.items()}

    d_xw_shard = nc.dram_tensor("xw_shard", [NSHARD, P], BF16)
    d_xw_full = nc.dram_tensor("xw_full", [N, P], BF16, addr_space="Shared")
    d_td = nc.dram_tensor("td_tab", [100, P], BF16)

    with tile.TileContext(nc) as tc:
        with (
            tc.tile_pool(name="const", bufs=1) as cpool,
            tc.tile_pool(name="resid", bufs=1) as rpool,
        ):
            C = {}
            for k, (sh, dt) in CSHAPE.items():
                C[k] = cpool.tile(sh, dt, tag=k, name=f"const_{k}")
                nc.sync.dma_start(out=C[k][:], in_=d_c[k].ap())
            nc.sync.dma_start(out=d_td.ap(), in_=d_c["c_Td"].ap())

            for _rep in range(REPS):
              xwt_s = rpool.tile([P, NSHARD], BF16, tag="xwt")  # x_w^T
              v2c_s = rpool.tile([P, NSHARD], BF16, tag="v2c")  # [node, w*feat]

              # ================= Phase 1: LLR fusion ==========================
              with (
                  tc.tile_pool(name="p1in", bufs=1) as sb1c,
                  tc.tile_pool(name="p1ps", bufs=2, space="PSUM") as pp1,
                  tc.tile_pool(name="p1pt", bufs=3, space="PSUM") as pp1t,
                  tc.tile_pool(name="p1sb", bufs=3) as sb1,
              ):
                  xt_s = sb1c.tile([P, NSHARD], BF16, tag="xt")
                  llr_t = sb1c.tile([1, NSHARD], BF16, tag="llr")
                  msk_t = sb1c.tile([1, NSHARD], BF16, tag="msk")
                  one_t = sb1c.tile([1, NSHARD], BF16, tag="one")
                  nc.sync.dma_start(out=xt_s[:], in_=d_xt.ap())
                  nc.sync.dma_start(out=llr_t[:], in_=d_lm.ap()[0:1, :])
                  nc.sync.dma_start(out=msk_t[:], in_=d_lm.ap()[1:2, :])
                  nc.sync.dma_start(out=one_t[:], in_=d_lm.ap()[2:3, :])

                  for g in range(NG1):
                      ns = slice(g * W1, (g + 1) * W1)
                      py = pp1.tile([P, W1], F32, tag="py")
                      nc.tensor.matmul(py[:], C["c_Wfx"][:], xt_s[:, ns],
                                       start=True, stop=False)
                      nc.tensor.matmul(py[:], C["c_wfl"][:], llr_t[0:1, ns],
                                       start=False, stop=False)
                      nc.tensor.matmul(py[:], C["c_bfr"][:], one_t[0:1, ns],
                                       start=False, stop=True)
                      yt = sb1.tile([P, W1], BF16, tag="yt")
                      nc.scalar.activation(yt[:], py[:], AF.Identity)
                      pn = pp1t.tile([P, B1, P], BF16, tag="pt")
                      for j in range(B1):
                          nc.tensor.transpose(pn[:, j, :],
                                              yt[:, j * P:(j + 1) * P],
                                              C["c_identb"][:])
                      st6 = sb1.tile([P, B1, 6], F32, tag="st6")
                      mv = sb1.tile([P, B1, 2], F32, tag="mv")
                      for j in range(B1):
                          nc.vector.bn_stats(out=st6[:, j, :], in_=pn[:, j, :])
                          nc.vector.bn_aggr(out=mv[:, j, :], in_=st6[:, j, :])
                      ve = sb1.tile([P, B1], F32, tag="ve")
                      nc.vector.tensor_scalar(out=ve[:], in0=mv[:, :, 1],
                                              scalar1=LN_EPS, scalar2=None,
                                              op0=OP.add)
                      sd = sb1.tile([P, B1], F32, tag="sd")
                      nc.scalar.activation(sd[:], ve[:], AF.Sqrt)
                      ivs = sb1.tile([P, B1], F32, tag="ivs")
                      nc.vector.reciprocal(ivs[:], sd[:])
                      nmi = sb1.tile([P, B1], F32, tag="nmi")
                      nc.vector.scalar_tensor_tensor(
                          out=nmi[:], in0=mv[:, :, 0], scalar=-1.0, in1=ivs[:],
                          op0=OP.mult, op1=OP.mult)
                      zn = sb1.tile([P, B1, P], BF16, tag="zn")
                      for j in range(B1):
                          nc.scalar.activation(zn[:, j, :], pn[:, j, :],
                                               AF.Identity,
                                               scale=ivs[:, j:j + 1],
                                               bias=nmi[:, j:j + 1])
                      pzt = pp1t.tile([P, B1, P], BF16, tag="pt")
                      for j in range(B1):
                          nc.tensor.transpose(pzt[:, j, :], zn[:, j, :],
                                              C["c_identb"][:])
                      fuT = sb1.tile([P, W1], BF16, tag="fuT")
                      nc.scalar.activation(
                          fuT[:], pzt[:].rearrange("p b f -> p (b f)"),
                          AF.Relu, scale=C["c_gfc"][:], bias=C["c_befc"][:])
                      pm = pp1.tile([P, W1], F32, tag="py")
                      nc.tensor.matmul(pm[:], C["c_ones1"][:], msk_t[0:1, ns],
                                       start=True, stop=True)
                      pmu = sb1.tile([P, W1], mybir.dt.uint8, tag="pmu")
                      nc.vector.tensor_copy(out=pmu[:], in_=pm[:])
                      nc.vector.tensor_copy(out=xwt_s[:, ns], in_=xt_s[:, ns])
                      nc.vector.copy_predicated(out=xwt_s[:, ns], mask=pmu[:],
                                                data=fuT[:])
                      pxw = pp1t.tile([P, B1, P], BF16, tag="pt")
                      for j in range(B1):
                          nc.tensor.transpose(
                              pxw[:, j, :],
                              xwt_s[:, g * W1 + j * P:g * W1 + (j + 1) * P],
                              C["c_identb"][:])
                      xw_nm = sb1.tile([P, B1, P], BF16, tag="xw_nm")
                      nc.scalar.activation(
                          xw_nm[:].rearrange("p b f -> p (b f)"),
                          pxw[:].rearrange("p b f -> p (b f)"), AF.Identity)
                      nc.sync.dma_start(
                          out=d_xw_shard.ap()[ns, :].rearrange(
                              "(b p) f -> p b f", p=P),
                          in_=xw_nm[:])
                      if PHASES == 1:
                          xwf = sb1.tile([P, B1, P], F32, tag="xwf")
                          nc.vector.tensor_copy(out=xwf[:], in_=xw_nm[:])
                          nc.sync.dma_start(
                              out=d_out.ap()[ns, :].rearrange(
                                  "(b p) f -> p b f", p=P),
                              in_=xwf[:])

              if PHASES >= 2 and not SKIP_AG:
                  nc.gpsimd.collective_compute(
                      "AllGather", OP.bypass,
                      replica_groups=[list(range(NCORES))],
                      ins=[d_xw_shard.ap().opt()],
                      outs=[d_xw_full.ap().opt()],
                  )

              # ================= Phase 2: edges ===============================
              with (
                  tc.tile_pool(name="pz", bufs=1, space="PSUM") as ppz,
                  tc.tile_pool(name="pxl", bufs=2, space="PSUM") as ppxl,
                  tc.tile_pool(name="po4", bufs=1, space="PSUM") as ppo4,
                  tc.tile_pool(name="pden", bufs=1, space="PSUM") as ppden,
                  tc.tile_pool(name="pal", bufs=1, space="PSUM") as ppal,
                  tc.tile_pool(name="ptrb", bufs=2, space="PSUM") as pptrb,
                  tc.tile_pool(name="e_in", bufs=2) as ein,
                  tc.tile_pool(name="e_c", bufs=1) as ecn,
                  tc.tile_pool(name="e_wk", bufs=3) as ewk,
                  tc.tile_pool(name="e_wk2", bufs=2) as ewk2,
              ):
                  dstc_t = None
                  if PHASES >= 2:
                      dstc_t = ecn.tile([P, TOTSUB], BF16, tag="dstc")
                      nc.sync.dma_start(out=dstc_t[:], in_=d_dstc.ap())
                  SMAX = max(S_prof) if S_prof else 1
                  for (w0, w1) in (cfg.chunks if PHASES >= 2 else []):
                      o0, o1 = offs[w0], offs[w1]
                      nsub = o1 - o0
                      idx_t = ein.tile([P, nsub], I32, tag="idx")
                      nc.sync.dma_start(out=idx_t[:], in_=d_idx.ap()[:, o0:o1])
                      dstr_t = ein.tile([1, nsub * P], BF16, tag="dstr")
                      nc.sync.dma_start(out=dstr_t[:],
                                        in_=d_dstr.ap()[:, o0 * P:o1 * P])
                      eat_t = ein.tile([8, nsub * P], BF16, tag="eat")
                      nc.sync.dma_start(out=eat_t[:],
                                        in_=d_eat.ap()[:, o0 * P:o1 * P])
                      xg_t = ein.tile([P, nsub, P], BF16, tag="xg")
                      if int(os.environ.get("GNN_SKIP_GATHER", "0")):
                          nc.sync.dma_start(
                              out=xg_t[:, 0:2, :],
                              in_=d_xw_full.ap()[0:2 * P, :].rearrange(
                                  "(s p) f -> p s f", p=P))
                      else:
                          for s in range(nsub):
                              nc.gpsimd.indirect_dma_start(
                                  out=xg_t[:, s, :], out_offset=None,
                                  in_=d_xw_full.ap(),
                                  in_offset=IndirectOffsetOnAxis(
                                      ap=idx_t[:, s:s + 1], axis=0),
                              )
                      for w in range(w0, w1):
                          S_w = S_prof[w]
                          lo = offs[w] - o0
                          nsw = slice(w * P, (w + 1) * P)
                          # one-hot matrices for this window
                          st_w = ewk.tile([P, SMAX * P], BF16, tag="stw")
                          for a in range(0, S_w, 4):
                              b = min(a + 4, S_w)
                              cols = (b - a) * P
                              pdst = pptrb.tile([P, 512], F32, tag="trb")
                              nc.tensor.matmul(
                                  pdst[:, :cols], C["c_ones1"][:],
                                  dstr_t[0:1, (lo + a) * P:(lo + b) * P],
                                  start=True, stop=True)
                              nc.vector.tensor_tensor(
                                  out=st_w[:, a * P:b * P],
                                  in0=_bc(C["c_iotap"][:], [P, cols]),
                                  in1=pdst[:, :cols], op=OP.is_equal)
                          S_wt = ewk.tile([P, SMAX, P], BF16, tag="Sw")
                          nc.vector.tensor_tensor(
                              out=S_wt[:, :S_w, :],
                              in0=_bc(dstc_t[:, offs[w]:offs[w] + S_w]
                                      .rearrange("p (s o) -> p s o", o=1),
                                      [P, S_w, P]),
                              in1=_bc(C["c_iota"][:].rearrange(
                                  "p (o f) -> p o f", o=1), [P, S_w, P]),
                              op=OP.is_equal)
                          pxr = ppz.tile([P, 512], F32, tag="z")
                          nc.tensor.matmul(pxr[:], xwt_s[:, nsw], C["c_Wr"][:],
                                           start=True, stop=True)
                          xr_sb = ewk2.tile([P, 512], BF16, tag="xr")
                          nc.scalar.activation(xr_sb[:], pxr[:], AF.Identity)
                          pden_t = ppden.tile([P, 4], F32, tag="den")
                          pden = pden_t[:]
                          aux = ppal.tile([P, 8], F32, tag="pal")
                          nc.tensor.matmul(aux[:, 4:8], xwt_s[:, nsw],
                                           C["c_wra"][:], start=True, stop=True)
                          xra_sb = ewk2.tile([P, 4], BF16, tag="xra")
                          nc.vector.tensor_copy(out=xra_sb[:], in_=aux[:, 4:8])
                          po4 = ppo4.tile([P, 512], F32, tag="o4")
                          ptg = pptrb.tile([P, S_w, P], BF16, tag="trb")
                          for ls in range(S_w):
                              nc.tensor.transpose(ptg[:, ls, :],
                                                  xg_t[:, lo + ls, :],
                                                  C["c_identb"][:])
                          xgT_w = ewk.tile([P, SMAX, P], BF16, tag="xgTw")
                          nc.scalar.activation(
                              xgT_w[:, :S_w, :].rearrange("p s f -> p (s f)"),
                              ptg[:].rearrange("p s f -> p (s f)"), AF.Identity)
                          for ls in range(S_w):
                              sl = lo + ls
                              xgT = xgT_w[:, ls, :]
                              St = st_w[:, ls * P:(ls + 1) * P]
                              Se = S_wt[:, ls, :]
                              ea_sl = eat_t[:, sl * P:(sl + 1) * P]
                              pz = ppz.tile([P, 512], F32, tag="z")
                              pxl = ppxl.tile([P, 512], F32, tag="xl")
                              pal = aux[:, 0:4]
                              nc.tensor.matmul(pz[:], xgT, C["c_Wl"][:],
                                               start=True, stop=False)
                              nc.tensor.matmul(pxl[:], xgT, C["c_Wl"][:],
                                               start=True, stop=True)
                              nc.tensor.matmul(pal, xgT, C["c_wla"][:],
                                               start=True, stop=False)
                              nc.tensor.matmul(pz[:], St, xr_sb[:],
                                               start=False, stop=False)
                              nc.tensor.matmul(pal, St, xra_sb[:],
                                               start=False, stop=False)
                              nc.tensor.matmul(pz[:], ea_sl, C["c_We"][:],
                                               start=False, stop=True)
                              nc.tensor.matmul(pal, ea_sl, C["c_wea"][:],
                                               start=False, stop=True)
                              # att.leaky(z) = att.z + 0.8 * att.relu(-z)
                              r = ewk.tile([P, 512], BF16, tag="r")
                              nc.scalar.activation(r[:], pz[:], AF.Relu,
                                                   scale=-1.0)
                              zat = ewk.tile([P, 512], BF16, tag="zat")
                              nc.vector.tensor_tensor(out=zat[:], in0=r[:],
                                                      in1=C["c_attb"][:],
                                                      op=OP.mult)
                              alr = ewk.tile([P, 4], F32, tag="alr")
                              nc.vector.reduce_sum(
                                  out=alr[:],
                                  in_=zat[:].rearrange("p (h c) -> p h c", h=4),
                                  axis=AX.X)
                              alpha = ewk.tile([P, 4], F32, tag="alpha")
                              nc.vector.scalar_tensor_tensor(
                                  out=alpha[:], in0=alr[:], scalar=0.8,
                                  in1=pal, op0=OP.mult, op1=OP.add)
                              au = ewk.tile([P, 4], BF16, tag="au")
                              nc.scalar.activation(au[:], alpha[:], AF.Exp)
                              nc.tensor.matmul(pden, Se, au[:],
                                               start=(ls == 0),
                                               stop=(ls == S_w - 1))
                              xla = ewk.tile([P, 4, P], BF16, tag="xla")
                              nc.vector.tensor_tensor(
                                  out=xla[:],
                                  in0=pxl[:].rearrange("p (h c) -> p h c", h=4),
                                  in1=_bc(au[:].rearrange("p (h o) -> p h o",
                                                          o=1), [P, 4, P]),
                                  op=OP.mult)
                              nc.tensor.matmul(
                                  po4[:], Se,
                                  xla[:].rearrange("p h c -> p (h c)"),
                                  start=(ls == 0), stop=(ls == S_w - 1))
                          dv = ewk.tile([P, 4], F32, tag="dv")
                          nc.vector.tensor_scalar(out=dv[:], in0=pden,
                                                  scalar1=SM_EPS, scalar2=None,
                                                  op0=OP.add)
                          iv = ewk.tile([P, 4], F32, tag="iv")
                          nc.vector.reciprocal(iv[:], dv[:])
                          nc.vector.tensor_scalar(out=iv[:], in0=iv[:],
                                                  scalar1=0.25, scalar2=None,
                                                  op0=OP.mult)
                          vacc = ewk2.tile([P, P], F32, tag="vacc")
                          nc.vector.tensor_scalar(
                              out=vacc[:], in0=po4[:, 0:P], scalar1=iv[:, 0:1],
                              scalar2=None, op0=OP.mult)
                          for h in range(1, 4):
                              hs = slice(h * P, (h + 1) * P)
                              nc.vector.scalar_tensor_tensor(
                                  out=vacc[:], in0=po4[:, hs],
                                  scalar=iv[:, h:h + 1], in1=vacc[:],
                                  op0=OP.mult, op1=OP.add)
                          nc.vector.tensor_copy(out=v2c_s[:, nsw],
                                                in_=vacc[:])

              if PHASES == 2:
                  with tc.tile_pool(name="dbg2", bufs=2) as dbg2:
                      for g in range(NG1):
                          ns = slice(g * W1, (g + 1) * W1)
                          vf = dbg2.tile([P, B1, P], F32, tag="vf")
                          nc.vector.tensor_copy(
                              out=vf[:],
                              in_=v2c_s[:, ns].rearrange("p (b f) -> p b f",
                                                         b=B1))
                          nc.sync.dma_start(
                              out=d_out.ap()[ns, :].rearrange(
                                  "(b p) f -> p b f", p=P),
                              in_=vf[:])

              # ================= Phase 3: degree gate + final LN ==============
              with (
                  tc.tile_pool(name="p3ps", bufs=2, space="PSUM") as pp3,
                  tc.tile_pool(name="p3pt", bufs=2, space="PSUM") as pp3t,
                  tc.tile_pool(name="g_in", bufs=1) as gin,
                  tc.tile_pool(name="g_big", bufs=1) as gbig,
                  tc.tile_pool(name="g_wk", bufs=3) as gwk,
              ):
                if PHASES >= 3:
                  degi = gin.tile([P, BPC], I32, tag="degi")
                  nc.sync.dma_start(out=degi[:], in_=d_deg.ap())
                  dterm = gbig.tile([P, BPC, P], BF16, tag="dterm")
                  for t in range(BPC):
                      nc.gpsimd.indirect_dma_start(
                          out=dterm[:, t, :], out_offset=None,
                          in_=d_td.ap(),
                          in_offset=IndirectOffsetOnAxis(
                              ap=degi[:, t:t + 1], axis=0),
                      )
                  h_all = gbig.tile([P, BPC, P], BF16, tag="h_all")
                  p_all = gbig.tile([P, BPC, P], BF16, tag="p_all")
                  mv1 = gin.tile([P, BPC, 2], F32, tag="mv1")
                  mv2 = gin.tile([P, BPC, 2], F32, tag="mv2")

                  def lnstats(src3, mv, gb0):
                      st6 = gwk.tile([P, B1, 6], F32, tag="st6")
                      for j in range(B1):
                          nc.vector.bn_stats(out=st6[:, j, :],
                                             in_=src3[:, j, :])
                          nc.vector.bn_aggr(out=mv[:, gb0 + j, :],
                                            in_=st6[:, j, :])

                  def invstd(mv, tag):
                      ve = gin.tile([P, BPC], F32, tag=tag + "e")
                      nc.vector.tensor_scalar(out=ve[:], in0=mv[:, :, 1],
                                              scalar1=LN_EPS, scalar2=None,
                                              op0=OP.add)
                      sd = gin.tile([P, BPC], F32, tag=tag + "s")
                      nc.scalar.activation(sd[:], ve[:], AF.Sqrt)
                      iv = gin.tile([P, BPC], F32, tag=tag + "i")
                      nc.vector.reciprocal(iv[:], sd[:])
                      nmi = gin.tile([P, BPC], F32, tag=tag + "n")
                      nc.vector.scalar_tensor_tensor(
                          out=nmi[:], in0=mv[:, :, 0], scalar=-1.0, in1=iv[:],
                          op0=OP.mult, op1=OP.mult)
                      return iv, nmi

                  # loop A: h_pre + stats
                  for g in range(NG1):
                      gb = slice(g * B1, (g + 1) * B1)
                      ptv = pp3t.tile([P, B1, P], BF16, tag="pt")
                      for j in range(B1):
                          nc.tensor.transpose(
                              ptv[:, j, :],
                              v2c_s[:, g * W1 + j * P:g * W1 + (j + 1) * P],
                              C["c_identb"][:])
                      v2cT = gwk.tile([P, W1], BF16, tag="v2cT")
                      nc.scalar.activation(
                          v2cT[:], ptv[:].rearrange("p b f -> p (b f)"),
                          AF.Identity)
                      ph4 = pp3.tile([P, W1], F32, tag="ps")
                      for j in range(B1):
                          nc.tensor.matmul(ph4[:, j * P:(j + 1) * P],
                                           v2cT[:, j * P:(j + 1) * P],
                                           C["c_Wg1x"][:], start=True,
                                           stop=True)
                      nc.vector.tensor_tensor(
                          out=h_all[:, gb, :],
                          in0=ph4[:].rearrange("p (b f) -> p b f", b=B1),
                          in1=dterm[:, gb, :], op=OP.add)
                      lnstats(h_all[:, gb, :], mv1, g * B1)
                  iv1, nmi1 = invstd(mv1, "i1")

                  # loop B: normalize -> gate matmul -> sigmoid -> p
                  for g in range(NG1):
                      gb = slice(g * B1, (g + 1) * B1)
                      zn = gwk.tile([P, B1, P], BF16, tag="zn")
                      for j in range(B1):
                          w = g * B1 + j
                          nc.scalar.activation(zn[:, j, :], h_all[:, w, :],
                                               AF.Identity,
                                               scale=iv1[:, w:w + 1],
                                               bias=nmi1[:, w:w + 1])
                      ptz = pp3t.tile([P, B1, P], BF16, tag="pt")
                      for j in range(B1):
                          nc.tensor.transpose(ptz[:, j, :], zn[:, j, :],
                                              C["c_identb"][:])
                      h2T = gwk.tile([P, W1], BF16, tag="h2T")
                      nc.scalar.activation(
                          h2T[:], ptz[:].rearrange("p b f -> p (b f)"),
                          AF.Relu, scale=C["c_gg"][:], bias=C["c_beg"][:])
                      pg4 = pp3.tile([P, W1], F32, tag="ps")
                      for j in range(B1):
                          nc.tensor.matmul(pg4[:, j * P:(j + 1) * P],
                                           h2T[:, j * P:(j + 1) * P],
                                           C["c_Wg2"][:], start=True,
                                           stop=False)
                          nc.tensor.matmul(pg4[:, j * P:(j + 1) * P],
                                           C["c_ones1"][:], C["c_bg2r"][:],
                                           start=False, stop=True)
                      gate = gwk.tile([P, B1, P], BF16, tag="gate")
                      nc.scalar.activation(
                          gate[:].rearrange("p b f -> p (b f)"), pg4[:],
                          AF.Sigmoid)
                      nc.vector.tensor_tensor(
                          out=p_all[:, gb, :],
                          in0=v2c_s[:, g * W1:(g + 1) * W1].rearrange(
                              "p (b f) -> p b f", b=B1),
                          in1=gate[:], op=OP.mult)
                      lnstats(p_all[:, gb, :], mv2, g * B1)
                  iv2, nmi2 = invstd(mv2, "i2")

                  # loop C: final LN + output
                  for g in range(NG1):
                      ns = slice(g * W1, (g + 1) * W1)
                      gb = slice(g * B1, (g + 1) * B1)
                      zc = gwk.tile([P, B1, P], BF16, tag="zc")
                      for j in range(B1):
                          w = g * B1 + j
                          nc.scalar.activation(zc[:, j, :], p_all[:, w, :],
                                               AF.Identity,
                                               scale=iv2[:, w:w + 1],
                                               bias=nmi2[:, w:w + 1])
                      t2 = gwk.tile([P, B1, P], F32, tag="t2")
                      nc.vector.tensor_tensor(
                          out=t2[:], in0=zc[:],
                          in1=_bc(C["c_gob"][:].rearrange("p (o f) -> p o f",
                                                          o=1), [P, B1, P]),
                          op=OP.mult)
                      y4 = gwk.tile([P, B1, P], F32, tag="y4")
                      nc.vector.tensor_tensor(
                          out=y4[:], in0=t2[:],
                          in1=_bc(C["c_bob"][:].rearrange("p (o f) -> p o f",
                                                          o=1), [P, B1, P]),
                          op=OP.add)
                      nc.sync.dma_start(
                          out=d_out.ap()[ns, :].rearrange("(b p) f -> p b f",
                                                          p=P),
                          in_=y4[:])

    nc.compile()
    return nc




def bench_hw(nc, in_maps, iters=32):
    """Build the sharded PJRT callable once; time repeated executions with all
    donated output buffers pre-staged on device."""
    import time
    import jax
    from jax.sharding import Mesh, PartitionSpec, NamedSharding
    from jax.experimental.shard_map import shard_map
    import concourse.mybir as mb
    from concourse import bass2jax as b2j

    b2j.install_neuronx_cc_hook()
    n_cores = len(in_maps)
    partition_name = (nc.partition_id_tensor.name
                      if nc.partition_id_tensor else None)
    in_names, out_names, out_avals, zero_outs = [], [], [], []
    for alloc in nc.m.functions[0].allocations:
        if not isinstance(alloc, mb.MemoryLocationSet):
            continue
        name = alloc.memorylocations[0].name
        if alloc.kind == "ExternalInput":
            if name != partition_name:
                in_names.append(name)
        elif alloc.kind == "ExternalOutput":
            out_names.append(name)
            shape = tuple(alloc.tensor_shape)
            dtype = mb.dt.np(alloc.dtype)
            out_avals.append(jax.core.ShapedArray(shape, dtype))
            zero_outs.append(np.zeros(shape, dtype))
    n_params = len(in_names)
    n_outs = len(out_avals)
    in_names.extend(out_names)
    if partition_name is not None:
        in_names.append(partition_name)
    donate = tuple(range(n_params, n_params + n_outs))

    def _body(*args):
        operands = list(args)
        if partition_name is not None:
            operands.append(b2j.partition_id_tensor())
        outs = b2j._bass_exec_p.bind(
            *operands,
            out_avals=tuple(out_avals), in_names=tuple(in_names),
            out_names=tuple(out_names), lowering_input_output_aliases=(),
            sim_require_finite=True, sim_require_nnan=True, nc=nc)
        return tuple(outs)

    devices = jax.devices()[:n_cores]
    mesh = Mesh(np.asarray(devices), ("core",))
    sharded = jax.jit(
        shard_map(_body, mesh=mesh,
                  in_specs=(PartitionSpec("core"),) * (n_params + n_outs),
                  out_specs=(PartitionSpec("core"),) * n_outs,
                  check_rep=False),
        donate_argnums=donate, keep_unused=True)

    concat_in = [
        np.concatenate([np.asarray(in_maps[c][in_names[i]])
                        for c in range(n_cores)], axis=0)
        for i in range(n_params)]
    sh = NamedSharding(mesh, PartitionSpec("core"))
    in_bufs = [jax.device_put(a, sh) for a in concat_in]

    def fresh_zeros():
        return [jax.device_put(
            np.zeros((n_cores * z.shape[0], *z.shape[1:]), z.dtype), sh)
            for z in zero_outs]

    out_arrs = sharded(*in_bufs, *fresh_zeros())
    jax.block_until_ready(out_arrs)
    results = [
        {name: np.asarray(out_arrs[i]).reshape(n_cores, *out_avals[i].shape)[c]
         for i, name in enumerate(out_names)}
        for c in range(n_cores)]

    zsets = [fresh_zeros() for _ in range(iters)]
    jax.block_until_ready(zsets)
    t0 = time.perf_counter()
    outs = [sharded(*in_bufs, *z) for z in zsets]
    jax.block_until_ready(outs)
    dt = (time.perf_counter() - t0) / iters
    return results, dt * 1e9


# ----------------------------------------------------------------------------
# Entry point
# ----------------------------------------------------------------------------

_CACHE = {}


def _get_kernel(cfg):
    key = cfg.key() + (os.environ.get("GNN_PHASES", "3"),
                       os.environ.get("GNN_SKIP_AG", "0"),
                       os.environ.get("GNN_SKIP_GATHER", "0"),
                       os.environ.get("GNN_BODY_REPS", "1"))
    if key not in _CACHE:
        _CACHE[key] = build_kernel(cfg)
    return _CACHE[key]


def kernel(**inputs):
    global LAST_EXEC_NS
    cfg, in_maps, p2o = host_prep(inputs)
    nc = _get_kernel(cfg)
    if bool(int(os.environ.get("GNN_BENCH", "0"))):
        results, ns = bench_hw(nc, in_maps,
                               iters=int(os.environ.get("GNN_ITERS", "32")))
        LAST_EXEC_NS = ns
    else:
        res = run_bass_kernel_spmd(nc, in_maps, core_ids=list(range(NCORES)))
        results = res.results
        LAST_EXEC_NS = res.exec_time_ns
    y_perm = np.concatenate([results[k]["y"] for k in range(NCORES)], axis=0)
    y = np.empty_like(y_perm)
    y[p2o] = y_perm
    return y.astype(np.float32)


LAST_EXEC_NS = None

